# revision 37
# baseline (speedup 1.0000x reference)
"""Multi-head cross-attention (MHAForCrossFusion) on 8 Trainium2 cores.

Sharding: core = (batch, head-group). Core 4*b+j owns batch b and heads
4j..4j+3 (CW=256 projection features). Each core reads only its batch's
q/k/v (host pre-transposed to feature-major, cast to bf16) and writes a
full-width fp32 partial of its batch's output rows; host sums 4 partials
per batch + bo.

Per-core device program (matmul operands bf16, fp32 PSUM accumulate):
 - all input DMAs issued up-front; K/Q projections feature-major
   [feat, tok]; V projected token-major (activation tile as the
   stationary operand) straight into the ones-augmented vma layout
   [key, head*(hv|1)] (softmax denominator trick)
 - attention per (512-query chunk lc, head-pair g): scores S.T = km.T @
   qm per head over each 128-key tile; the two heads of the pair are
   row-packed in the PE array (tile_position) and land in adjacent PSUM
   banks, so exp runs as one ACT instruction over [128, 2*512]
 - ctx_aug[0:65] = [vm | 1].T @ expS accumulated over key tiles;
   row 64 = softmax denominator
 - attention starts after only the first k/v/q chunks are projected;
   remaining projections, normalizes and out-projections are deferred
   "jobs" drained inside later key loops so ACT (the bottleneck engine)
   never waits on a cold pipeline
 - normalize: ctx evacuated to SBUF (releases the PSUM bank), 1/d as
   exp(-ln(d)) on ACT (Ln/Exp share a table set; runs in ACT's
   chunk-boundary idle window), K=1 matmul broadcast across partitions,
   DVE multiply -> ctxn
 - out-projection: out[t, :] = ctxn.T @ Wo_slice.T (partial sum), with
   the last chunk's normalize/out-projection quarter-pipelined to
   shorten the kernel tail
"""

import os
from collections import deque

import numpy as np
import ml_dtypes

import concourse.bass as bass
import concourse.mybir as mybir
import concourse.tile as tile
from concourse import bass_utils

N_CORES = 8
B, L, D = 2, 2048, 1024
NH, HD = 16, 64
HG = NH // (N_CORES // B)  # 4 heads per core
CW = HG * HD  # 256 projection features per core
SCALE = 1.0 / np.sqrt(HD)

F32 = mybir.dt.float32
BF16 = mybir.dt.bfloat16

DC = D // 128  # 8 contraction tiles for the projections
NT = L // 128  # 16 key tiles
NCH = L // 512  # 4 token chunks


def _split_matmul_waits(nc):
    """fp32/fp32r matmuls lower to a self-loading LDW whose ISA struct has a
    single sem-wait slot (HWDGE DMA likewise); walrus rejects >1 wait. Move
    extra waits onto same-engine NoOps inserted right before the matmul
    (program order on the sequencer preserves the happens-before)."""
    for f in nc.m.functions:
        for bb in f.blocks:
            insts = list(bb.instructions)
            out = []
            for inst in insts:
                si = inst.sync_info
                if si is not None and len(si.on_wait) > 1:
                    for w in si.on_wait[:-1]:
                        nop = mybir.InstNoOp(
                            name=nc.get_next_instruction_name(),
                            ins=[],
                            outs=[],
                            engine=inst.engine,
                            bass_nofuse=True,
                        )
                        nop.sync_info = mybir.SyncInfo(on_wait=[w], on_update=[])
                        out.append(nop)
                    inst.sync_info = mybir.SyncInfo(
                        on_wait=[si.on_wait[-1]], on_update=si.on_update
                    )
                out.append(inst)
            if len(out) != len(insts):
                bb.instructions = out
    return nc


def build_nc():
    nc = bass.Bass("TRN2", target_bir_lowering=False, debug=False)

    qT = nc.dram_tensor("qT", [D, L], BF16, kind="ExternalInput").ap()
    kT = nc.dram_tensor("kT", [D, L], BF16, kind="ExternalInput").ap()
    vT = nc.dram_tensor("vT", [D, L], BF16, kind="ExternalInput").ap()
    wqt = nc.dram_tensor("wqt", [D, CW], BF16, kind="ExternalInput").ap()
    wkt = nc.dram_tensor("wkt", [D, CW], BF16, kind="ExternalInput").ap()
    wvt = nc.dram_tensor("wvt", [D, CW], BF16, kind="ExternalInput").ap()
    wot = nc.dram_tensor("wot", [CW, D], BF16, kind="ExternalInput").ap()
    bq = nc.dram_tensor("bq", [CW, 1], F32, kind="ExternalInput").ap()
    bk = nc.dram_tensor("bk", [CW, 1], F32, kind="ExternalInput").ap()
    bvb = nc.dram_tensor("bvb", [128, CW], F32, kind="ExternalInput").ap()
    out_p = nc.dram_tensor("out_p", [L, D], F32, kind="ExternalOutput").ap()

    with tile.TileContext(nc) as tc:
        with (
            tc.tile_pool(name="singles", bufs=1) as singles,
            tc.tile_pool(name="acts", bufs=1) as acts,
            tc.tile_pool(name="stage", bufs=12) as stage,
            tc.tile_pool(name="small", bufs=4) as small,
            tc.tile_pool(name="norm", bufs=3) as normp,
            tc.tile_pool(name="esp", bufs=6) as esp,
            tc.tile_pool(name="psq", bufs=2, space="PSUM") as ppq,
            tc.tile_pool(name="psa", bufs=2, space="PSUM") as ppa,
            tc.tile_pool(name="psc", bufs=2, space="PSUM") as ppc,
        ):
            ones = singles.tile([1, 64], BF16)
            nc.vector.memset(ones, 1.0)

            b_sb = {}
            for name, dram in (("bq", bq), ("bk", bk)):
                b = singles.tile([128, 2], F32, name=name + "_sb")
                nc.sync.dma_start(b, dram.rearrange("(g p) one -> p (g one)", p=128))
                b_sb[name] = b
            bvb_sb = singles.tile([128, CW], F32)
            nc.sync.dma_start(bvb_sb, bvb)
            w_sb = {}
            for name, dram in (("wq", wqt), ("wk", wkt), ("wv", wvt)):
                w_sb[name] = singles.tile([128, DC, CW], BF16, name=name + "_sb")
            nc.sync.dma_start(w_sb["wk"], wkt.rearrange("(c p) h -> p c h", p=128))
            nc.scalar.dma_start(w_sb["wq"], wqt.rearrange("(c p) h -> p c h", p=128))

            # q/k/v chunk DMAs up-front, ordered so the first attention
            # chunk's inputs (k0, q0, v0) land first
            xcs = {}
            dmaq = [("k", 0), ("q", 0), ("v", 0)]
            for ci in range(1, NCH):
                dmaq += [("k", ci), ("v", ci)]
            for ci in range(1, NCH):
                dmaq += [("q", ci)]
            drams = {"k": kT, "v": vT, "q": qT}
            for idx, (name, ci) in enumerate(dmaq):
                xc = stage.tile([128, DC, 512], BF16, tag="xc", name=f"{name}c{ci}")
                eng = nc.scalar if name == "q" else nc.sync
                eng.dma_start(
                    xc,
                    drams[name].rearrange("(c p) t -> p c t", p=128)[
                        :, :, ci * 512 : (ci + 1) * 512
                    ],
                )
                xcs[name, ci] = xc
                if idx == 1:
                    nc.sync.dma_start(
                        w_sb["wv"], wvt.rearrange("(c p) h -> p c h", p=128)
                    )
            wot_sb = singles.tile([128, 2, D], BF16)
            nc.sync.dma_start(wot_sb, wot.rearrange("(g p) d -> p g d", p=128))

            qm = acts.tile([128, 2, L], BF16)  # feature-major projections
            km = acts.tile([128, 2, L], BF16)
            vma = acts.tile([128, NT, HG * 66], BF16)  # [key, kt, (hv|1) x4]
            ctxn = acts.tile([128, 2, L], BF16)

            # ones columns of the augmented V (col 64 of each head's 66)
            nc.vector.memset(
                vma.rearrange("p t (h c) -> p t h c", c=66)[:, :, :, 64], 1.0
            )

            # warm the ACT exp table set during phase 1 instead of paying
            # the ~2.7us PSEUDO_LOAD at the first real exp
            dummy = singles.tile([128, 2], F32)
            nc.vector.memset(dummy[:, 0:1], 0.0)
            nc.scalar.activation(
                dummy[:, 1:2], dummy[:, 0:1], mybir.ActivationFunctionType.Exp
            )
            # ... and warm the PE HAM clock gate with throwaway matmuls that
            # run while the first input DMAs are still in flight
            wu = singles.tile([1, 512], BF16)
            nc.vector.memset(wu, 0.0)
            for i in range(12):
                wps = ppa.tile([128, 512], F32, tag="aux")
                nc.tensor.matmul(wps[0:64, :], lhsT=ones, rhs=wu)

            def fproj(name, ci, g, part=None, pool="aux"):
                # feature-major projection of one 512-token chunk (k or q);
                # part=(ps, 0|1) splits the 8-dc accumulation into two jobs
                dst, bias = (km, b_sb["bk"]) if name == "k" else (qm, b_sb["bq"])
                ts = slice(ci * 512, (ci + 1) * 512)
                ps, h0, h1 = part if part else (None, 0, DC)
                if ps is None:
                    if pool == "aux":
                        ps = ppa.tile([128, 512], F32, tag="aux", name="fp_ps")
                    else:
                        ps = ppq.tile(
                            [128, 2, 512], F32, tag="sq", name="fp_ps"
                        )[:, 0, :]
                for dc in range(h0, h1):
                    nc.tensor.matmul(
                        ps,
                        lhsT=w_sb["w" + name][:, dc, g * 128 : (g + 1) * 128],
                        rhs=xcs[name, ci][:, dc, :],
                        start=(dc == 0),
                        stop=(dc == DC - 1),
                    )
                if h1 == DC:
                    nc.vector.tensor_scalar_add(
                        dst[:, g, ts], ps, bias[:, g : g + 1]
                    )
                return ps

            def qproj_jobs(ci):
                state = {}
                def a(ci=ci, g=None):
                    state[g] = fproj("q", ci, g, part=(None, 0, DC // 2))
                def b(ci=ci, g=None):
                    fproj("q", ci, g, part=(state[g], DC // 2, DC))
                out = []
                for g in range(2):
                    out.append(lambda g=g: a(g=g))
                    out.append(lambda g=g: b(g=g))
                return out

            def vproj(ci, tt):
                # token-major projection: vm[t, f], activations stationary;
                # runs inside the lc=0 window where aux holds g1's ctx, so
                # borrow a squad slot
                ps = ppa.tile([128, 512], F32, tag="aux", name="vp_ps")
                for dc in range(DC):
                    nc.tensor.matmul(
                        ps[:, 0:CW],
                        lhsT=xcs["v", ci][:, dc, tt * 128 : (tt + 1) * 128],
                        rhs=w_sb["wv"][:, dc, :],
                        start=(dc == 0),
                        stop=(dc == DC - 1),
                    )
                nc.vector.tensor_add(
                    vma.rearrange("p t (h c) -> p t h c", c=66)[
                        :, ci * 4 + tt, :, 0:64
                    ],
                    ps[:, 0:CW].rearrange("p (h c) -> p h c", c=64),
                    bvb_sb.rearrange("p (h c) -> p h c", c=64),
                )

            def emit_normalize(pg, lc, h, cu, rc, q0=0, q1=512):
                # normalize tokens [q0:q1) of head h in chunk lc
                po = ppa.tile([128, 512], F32, tag="aux")
                nc.tensor.matmul(
                    po[0:64, 0 : q1 - q0], lhsT=ones, rhs=rc[:, h, q0:q1]
                )
                nc.vector.tensor_mul(
                    ctxn[h * 64 : (h + 1) * 64, pg, lc * 512 + q0 : lc * 512 + q1],
                    cu[0:64, h, q0:q1],
                    po[0:64, 0 : q1 - q0],
                )

            def emit_outproj(lc, tt):
                t0 = lc * 512 + tt * 128
                ob = small.tile([128, D], F32, tag="ob")
                for half in range(2):
                    po = ppa.tile([128, 512], F32, tag="aux")
                    for g in range(2):
                        nc.tensor.matmul(
                            po,
                            lhsT=ctxn[:, g, t0 : t0 + 128],
                            rhs=wot_sb[:, g, half * 512 : (half + 1) * 512],
                            start=(g == 0),
                            stop=(g == 1),
                        )
                    nc.vector.tensor_copy(ob[:, half * 512 : (half + 1) * 512], po)
                nc.sync.dma_start(out_p[t0 : t0 + 128, :], ob)

            # phase 1 head: just enough to start attention; everything
            # else becomes a (ready_tick, fn) job drained inside the key
            # loops. ready_tick delays normalize/out-proj jobs so their PE
            # instructions never wait on DVE latency in the in-order stream.
            for g in range(2):
                fproj("k", 0, g)
            for g in range(2):
                fproj("q", 0, g)
            jobs = deque()
            for tt in range(4):
                jobs.append((0, lambda tt=tt: vproj(0, tt)))
            for ci in range(1, NCH):
                for g in range(2):
                    jobs.append((0, lambda ci=ci, g=g: fproj("k", ci, g)))
                for tt in range(4):
                    jobs.append((0, lambda ci=ci, tt=tt: vproj(ci, tt)))

            # ---- phase 2: attention; deferred jobs drain inside key loops
            for lc in range(NCH):
                ls = slice(lc * 512, (lc + 1) * 512)
                for g in range(2):
                    ctx = [
                        ppc.tile([128, 512], F32, tag="ctx", name=f"ctx{h}")
                        for h in range(2)
                    ]
                    cidx = lc * 2 + g
                    for pt in range(NT):
                        # drain deferred jobs BEFORE this pt's consumers are
                        # traced: a vma/km/qm write traced after its reader
                        # would not be seen as a RAW dependency by Tile
                        tick = cidx * NT + pt
                        budget = 2 if cidx == 0 else 1
                        while budget and jobs and jobs[0][0] <= tick:
                            jobs.popleft()[1]()
                            budget -= 1
                        ks = slice(pt * 128, (pt + 1) * 128)
                        squad = ppq.tile([128, 2, 512], F32, tag="sq")
                        for h in range(2):
                            hs = slice(h * 64, (h + 1) * 64)
                            nc.tensor.matmul(
                                squad[:, h, :],
                                lhsT=km[hs, g, ks],
                                rhs=qm[hs, g, ls],
                                tile_position=(h * 64, 0),
                            )
                        es = esp.tile([128, 2, 512], BF16, tag="es")
                        nc.scalar.activation(
                            es.rearrange("p a b -> p (a b)"),
                            squad.rearrange("p a b -> p (a b)"),
                            mybir.ActivationFunctionType.Exp,
                            scale=SCALE,
                        )
                        for h in range(2):
                            nc.tensor.matmul(
                                ctx[h][0:65, :],
                                lhsT=vma[
                                    :, pt, (g * 2 + h) * 66 : (g * 2 + h) * 66 + 65
                                ],
                                rhs=es[:, h, :],
                                start=(pt == 0),
                                stop=(pt == NT - 1),
                            )
                    last = lc == NCH - 1 and g == 1
                    # evacuate ctx to SBUF (frees the PSUM banks) and start
                    # the reciprocal; the normalize runs as a later job
                    cu = normp.tile([65, 2, 512], F32, tag="cu")
                    if last:
                        # quartered so the tail chain starts sooner
                        for qtr in range(4):
                            for h in range(2):
                                nc.vector.tensor_copy(
                                    cu[:, h, qtr * 128 : (qtr + 1) * 128],
                                    ctx[h][0:65, qtr * 128 : (qtr + 1) * 128],
                                )
                    else:
                        for h in range(2):
                            nc.vector.tensor_copy(cu[:, h, :], ctx[h][0:65, :])
                    # 1/d as exp(-ln(d)) on ACT: Ln and Exp share one table
                    # set, this runs in ACT's chunk-boundary idle window
                    # instead of 3.9us on the DVE, and both heads' rows are
                    # adjacent so one instruction pair covers them
                    lnd = normp.tile([1, 2, 512], F32, tag="lnd")
                    rc = normp.tile([1, 2, 512], BF16, tag="rc")
                    if not last:
                        nc.scalar.activation(
                            lnd, cu[64:65, :, :], mybir.ActivationFunctionType.Ln
                        )
                        nc.scalar.activation(
                            rc, lnd, mybir.ActivationFunctionType.Exp, scale=-1.0
                        )
                    nxt = (cidx + 1) * NT
                    if g == 0 and lc < NCH - 1:
                        for fn in qproj_jobs(lc + 1):
                            jobs.append((0, fn))
                    if not last:
                        for h in range(2):
                            jobs.append(
                                (
                                    nxt + 4 + h,
                                    lambda g=g, lc=lc, h=h, cu=cu, rc=rc: (
                                        emit_normalize(g, lc, h, cu, rc)
                                    ),
                                )
                            )
                    if g == 1 and lc < NCH - 1:
                        for tt in range(4):
                            jobs.append(
                                (
                                    nxt + 7 + tt,
                                    lambda lc=lc, tt=tt: emit_outproj(lc, tt),
                                )
                            )
            while jobs:
                jobs.popleft()[1]()
            # tail: quarter-pipelined normalize + out-projection of the
            # last chunk so the reciprocal latency overlaps the out DMAs
            for qtr in range(4):
                q0, q1 = qtr * 128, (qtr + 1) * 128
                nc.scalar.activation(
                    lnd[:, :, q0:q1],
                    cu[64:65, :, q0:q1],
                    mybir.ActivationFunctionType.Ln,
                )
                nc.scalar.activation(
                    rc[:, :, q0:q1],
                    lnd[:, :, q0:q1],
                    mybir.ActivationFunctionType.Exp,
                    scale=-1.0,
                )
                for h in range(2):
                    emit_normalize(1, NCH - 1, h, cu, rc, q0, q1)
                emit_outproj(NCH - 1, qtr)
    return _split_matmul_waits(nc)


_NC_CACHE = None


def kernel(q, k, v, attention_mask, Wq, bq, Wk, bk, Wv, bv, Wo, bo):
    global _NC_CACHE
    q, k, v = (np.asarray(x, np.float32) for x in (q, k, v))
    assert np.asarray(attention_mask).all(), "kernel assumes all-ones mask"
    if _NC_CACHE is None:
        _NC_CACHE = build_nc()
    nc = _NC_CACHE

    bfc = lambda x: np.ascontiguousarray(np.asarray(x, ml_dtypes.bfloat16))
    c = np.ascontiguousarray
    Wq, Wk, Wv, Wo = (np.asarray(x, np.float32) for x in (Wq, Wk, Wv, Wo))
    bq, bk, bv, bo = (np.asarray(x, np.float32) for x in (bq, bk, bv, bo))

    qT = [bfc(q[b].T) for b in range(B)]
    kT = [bfc(k[b].T) for b in range(B)]
    vT = [bfc(v[b].T) for b in range(B)]

    in_maps = []
    for ci in range(N_CORES):
        b = ci // (N_CORES // B)
        j = ci % (N_CORES // B)
        hs = slice(j * CW, (j + 1) * CW)
        in_maps.append(
            {
                "qT": qT[b],
                "kT": kT[b],
                "vT": vT[b],
                "wqt": bfc(Wq.T[:, hs]),
                "wkt": bfc(Wk.T[:, hs]),
                "wvt": bfc(Wv.T[:, hs]),
                "wot": bfc(Wo.T[hs, :]),
                "bq": c(bq[hs, None]),
                "bk": c(bk[hs, None]),
                "bvb": c(np.broadcast_to(bv[hs][None, :], (128, CW))),
            }
        )

    res = bass_utils.run_bass_kernel_spmd(
        nc,
        in_maps,
        core_ids=list(range(N_CORES)),
        tmpdir=os.environ.get("KERNEL_TMPDIR"),
    )
    globals()["LAST_RES"] = res
    out = np.zeros((B, L, D), np.float32)
    for ci, r in enumerate(res.results):
        out[ci // (N_CORES // B)] += r["out_p"]
    out += bo[None, None, :]
    return out


# revision 38
# speedup vs baseline: 1.0132x; 1.0132x over previous
"""Multi-head cross-attention (MHAForCrossFusion) on 8 Trainium2 cores.

Sharding: core = (batch, head-group). Core 4*b+j owns batch b and heads
4j..4j+3 (CW=256 projection features). Each core reads only its batch's
q/k/v (host pre-transposed to feature-major, cast to bf16) and writes a
full-width fp32 partial of its batch's output rows; host sums 4 partials
per batch + bo.

Per-core device program (matmul operands bf16, fp32 PSUM accumulate):
 - all input DMAs issued up-front; K/Q projections feature-major
   [feat, tok]; V projected token-major (activation tile as the
   stationary operand) straight into the ones-augmented vma layout
   [key, head*(hv|1)] (softmax denominator trick)
 - attention per (512-query chunk lc, head-pair g): scores S.T = km.T @
   qm per head over each 128-key tile; the two heads of the pair are
   row-packed in the PE array (tile_position) and land in adjacent PSUM
   banks, so exp runs as one ACT instruction over [128, 2*512]
 - ctx_aug[0:65] = [vm | 1].T @ expS accumulated over key tiles;
   row 64 = softmax denominator
 - attention starts after only the first k/v/q chunks are projected;
   remaining projections, normalizes and out-projections are deferred
   "jobs" drained inside later key loops so ACT (the bottleneck engine)
   never waits on a cold pipeline
 - normalize: ctx evacuated to SBUF (releases the PSUM bank), 1/d as
   exp(-ln(d)) on ACT (Ln/Exp share a table set; runs in ACT's
   chunk-boundary idle window), K=1 matmul broadcast across partitions,
   DVE multiply -> ctxn
 - out-projection: out[t, :] = ctxn.T @ Wo_slice.T (partial sum), with
   the last chunk's normalize/out-projection quarter-pipelined to
   shorten the kernel tail
"""

import os
from collections import deque

import numpy as np
import ml_dtypes

import concourse.bass as bass
import concourse.mybir as mybir
import concourse.tile as tile
from concourse import bass_utils

N_CORES = 8
B, L, D = 2, 2048, 1024
NH, HD = 16, 64
HG = NH // (N_CORES // B)  # 4 heads per core
CW = HG * HD  # 256 projection features per core
SCALE = 1.0 / np.sqrt(HD)

F32 = mybir.dt.float32
BF16 = mybir.dt.bfloat16

DC = D // 128  # 8 contraction tiles for the projections
NT = L // 128  # 16 key tiles
NCH = L // 512  # 4 token chunks


def _split_matmul_waits(nc):
    """fp32/fp32r matmuls lower to a self-loading LDW whose ISA struct has a
    single sem-wait slot (HWDGE DMA likewise); walrus rejects >1 wait. Move
    extra waits onto same-engine NoOps inserted right before the matmul
    (program order on the sequencer preserves the happens-before)."""
    for f in nc.m.functions:
        for bb in f.blocks:
            insts = list(bb.instructions)
            out = []
            for inst in insts:
                si = inst.sync_info
                if si is not None and len(si.on_wait) > 1:
                    for w in si.on_wait[:-1]:
                        nop = mybir.InstNoOp(
                            name=nc.get_next_instruction_name(),
                            ins=[],
                            outs=[],
                            engine=inst.engine,
                            bass_nofuse=True,
                        )
                        nop.sync_info = mybir.SyncInfo(on_wait=[w], on_update=[])
                        out.append(nop)
                    inst.sync_info = mybir.SyncInfo(
                        on_wait=[si.on_wait[-1]], on_update=si.on_update
                    )
                out.append(inst)
            if len(out) != len(insts):
                bb.instructions = out
    return nc


def build_nc():
    nc = bass.Bass("TRN2", target_bir_lowering=False, debug=False)

    qT = nc.dram_tensor("qT", [D, L], BF16, kind="ExternalInput").ap()
    kT = nc.dram_tensor("kT", [D, L], BF16, kind="ExternalInput").ap()
    vT = nc.dram_tensor("vT", [D, L], BF16, kind="ExternalInput").ap()
    wqt = nc.dram_tensor("wqt", [D, CW], BF16, kind="ExternalInput").ap()
    wkt = nc.dram_tensor("wkt", [D, CW], BF16, kind="ExternalInput").ap()
    wvt = nc.dram_tensor("wvt", [D, CW], BF16, kind="ExternalInput").ap()
    wot = nc.dram_tensor("wot", [CW, D], BF16, kind="ExternalInput").ap()
    bq = nc.dram_tensor("bq", [CW, 1], F32, kind="ExternalInput").ap()
    bk = nc.dram_tensor("bk", [CW, 1], F32, kind="ExternalInput").ap()
    bvb = nc.dram_tensor("bvb", [128, CW], F32, kind="ExternalInput").ap()
    out_p = nc.dram_tensor("out_p", [L, D], F32, kind="ExternalOutput").ap()

    with tile.TileContext(nc) as tc:
        with (
            tc.tile_pool(name="singles", bufs=1) as singles,
            tc.tile_pool(name="acts", bufs=1) as acts,
            tc.tile_pool(name="stage", bufs=12) as stage,
            tc.tile_pool(name="small", bufs=4) as small,
            tc.tile_pool(name="norm", bufs=3) as normp,
            tc.tile_pool(name="esp", bufs=6) as esp,
            tc.tile_pool(name="psq", bufs=2, space="PSUM") as ppq,
            tc.tile_pool(name="psa", bufs=2, space="PSUM") as ppa,
            tc.tile_pool(name="psc", bufs=2, space="PSUM") as ppc,
        ):
            ones = singles.tile([1, 64], BF16)
            nc.vector.memset(ones, 1.0)

            b_sb = {}
            for name, dram in (("bq", bq), ("bk", bk)):
                b = singles.tile([128, 2], F32, name=name + "_sb")
                nc.sync.dma_start(b, dram.rearrange("(g p) one -> p (g one)", p=128))
                b_sb[name] = b
            bvb_sb = singles.tile([128, CW], F32)
            nc.sync.dma_start(bvb_sb, bvb)
            w_sb = {}
            for name, dram in (("wq", wqt), ("wk", wkt), ("wv", wvt)):
                w_sb[name] = singles.tile([128, DC, CW], BF16, name=name + "_sb")
            nc.sync.dma_start(w_sb["wk"], wkt.rearrange("(c p) h -> p c h", p=128))
            nc.sync.dma_start(w_sb["wq"], wqt.rearrange("(c p) h -> p c h", p=128))

            # q/k/v chunk DMAs up-front, ordered so the first attention
            # chunk's inputs (k0, q0, v0) land first
            xcs = {}
            dmaq = [("k", 0), ("q", 0), ("v", 0)]
            for ci in range(1, NCH):
                dmaq += [("k", ci), ("v", ci)]
            for ci in range(1, NCH):
                dmaq += [("q", ci)]
            drams = {"k": kT, "v": vT, "q": qT}
            for idx, (name, ci) in enumerate(dmaq):
                xc = stage.tile([128, DC, 512], BF16, tag="xc", name=f"{name}c{ci}")
                nc.sync.dma_start(
                    xc,
                    drams[name].rearrange("(c p) t -> p c t", p=128)[
                        :, :, ci * 512 : (ci + 1) * 512
                    ],
                )
                xcs[name, ci] = xc
                if idx == 1:
                    nc.sync.dma_start(
                        w_sb["wv"], wvt.rearrange("(c p) h -> p c h", p=128)
                    )
            wot_sb = singles.tile([128, 2, D], BF16)
            nc.sync.dma_start(wot_sb, wot.rearrange("(g p) d -> p g d", p=128))

            qm = acts.tile([128, 2, L], BF16)  # feature-major projections
            km = acts.tile([128, 2, L], BF16)
            vma = acts.tile([128, NT, HG * 66], BF16)  # [key, kt, (hv|1) x4]
            ctxn = acts.tile([128, 2, L], BF16)

            # ones columns of the augmented V (col 64 of each head's 66)
            nc.vector.memset(
                vma.rearrange("p t (h c) -> p t h c", c=66)[:, :, :, 64], 1.0
            )

            # warm the ACT exp table set during phase 1 instead of paying
            # the ~2.7us PSEUDO_LOAD at the first real exp
            dummy = singles.tile([128, 2], F32)
            nc.vector.memset(dummy[:, 0:1], 0.0)
            nc.scalar.activation(
                dummy[:, 1:2], dummy[:, 0:1], mybir.ActivationFunctionType.Exp
            )
            # ... and warm the PE HAM clock gate with throwaway matmuls that
            # run while the first input DMAs are still in flight
            wu = singles.tile([1, 512], BF16)
            nc.vector.memset(wu, 0.0)
            for i in range(12):
                wps = ppa.tile([128, 512], F32, tag="aux")
                nc.tensor.matmul(wps[0:64, :], lhsT=ones, rhs=wu)

            def fproj(name, ci, g, part=None, pool="aux"):
                # feature-major projection of one 512-token chunk (k or q);
                # part=(ps, 0|1) splits the 8-dc accumulation into two jobs
                dst, bias = (km, b_sb["bk"]) if name == "k" else (qm, b_sb["bq"])
                ts = slice(ci * 512, (ci + 1) * 512)
                ps, h0, h1 = part if part else (None, 0, DC)
                if ps is None:
                    if pool == "aux":
                        ps = ppa.tile([128, 512], F32, tag="aux", name="fp_ps")
                    else:
                        ps = ppq.tile(
                            [128, 2, 512], F32, tag="sq", name="fp_ps"
                        )[:, 0, :]
                for dc in range(h0, h1):
                    nc.tensor.matmul(
                        ps,
                        lhsT=w_sb["w" + name][:, dc, g * 128 : (g + 1) * 128],
                        rhs=xcs[name, ci][:, dc, :],
                        start=(dc == 0),
                        stop=(dc == DC - 1),
                    )
                if h1 == DC:
                    nc.vector.tensor_scalar_add(
                        dst[:, g, ts], ps, bias[:, g : g + 1]
                    )
                return ps

            def qproj_jobs(ci):
                state = {}
                def a(ci=ci, g=None):
                    state[g] = fproj("q", ci, g, part=(None, 0, DC // 2))
                def b(ci=ci, g=None):
                    fproj("q", ci, g, part=(state[g], DC // 2, DC))
                out = []
                for g in range(2):
                    out.append(lambda g=g: a(g=g))
                    out.append(lambda g=g: b(g=g))
                return out

            def vproj(ci, tt):
                # token-major projection: vm[t, f], activations stationary;
                # runs inside the lc=0 window where aux holds g1's ctx, so
                # borrow a squad slot
                ps = ppa.tile([128, 512], F32, tag="aux", name="vp_ps")
                for dc in range(DC):
                    nc.tensor.matmul(
                        ps[:, 0:CW],
                        lhsT=xcs["v", ci][:, dc, tt * 128 : (tt + 1) * 128],
                        rhs=w_sb["wv"][:, dc, :],
                        start=(dc == 0),
                        stop=(dc == DC - 1),
                    )
                nc.vector.tensor_add(
                    vma.rearrange("p t (h c) -> p t h c", c=66)[
                        :, ci * 4 + tt, :, 0:64
                    ],
                    ps[:, 0:CW].rearrange("p (h c) -> p h c", c=64),
                    bvb_sb.rearrange("p (h c) -> p h c", c=64),
                )

            def emit_normalize(pg, lc, h, cu, rc, q0=0, q1=512):
                # normalize tokens [q0:q1) of head h in chunk lc
                po = ppa.tile([128, 512], F32, tag="aux")
                nc.tensor.matmul(
                    po[0:64, 0 : q1 - q0], lhsT=ones, rhs=rc[:, h, q0:q1]
                )
                nc.vector.tensor_mul(
                    ctxn[h * 64 : (h + 1) * 64, pg, lc * 512 + q0 : lc * 512 + q1],
                    cu[0:64, h, q0:q1],
                    po[0:64, 0 : q1 - q0],
                )

            def emit_outproj(lc, tt):
                t0 = lc * 512 + tt * 128
                ob = small.tile([128, D], F32, tag="ob")
                for half in range(2):
                    po = ppa.tile([128, 512], F32, tag="aux")
                    for g in range(2):
                        nc.tensor.matmul(
                            po,
                            lhsT=ctxn[:, g, t0 : t0 + 128],
                            rhs=wot_sb[:, g, half * 512 : (half + 1) * 512],
                            start=(g == 0),
                            stop=(g == 1),
                        )
                    nc.vector.tensor_copy(ob[:, half * 512 : (half + 1) * 512], po)
                nc.sync.dma_start(out_p[t0 : t0 + 128, :], ob)

            # phase 1 head: just enough to start attention; everything
            # else becomes a (ready_tick, fn) job drained inside the key
            # loops. ready_tick delays normalize/out-proj jobs so their PE
            # instructions never wait on DVE latency in the in-order stream.
            for g in range(2):
                fproj("k", 0, g)
            for g in range(2):
                fproj("q", 0, g)
            jobs = deque()
            for tt in range(4):
                jobs.append((0, lambda tt=tt: vproj(0, tt)))
            for ci in range(1, NCH):
                for g in range(2):
                    jobs.append((0, lambda ci=ci, g=g: fproj("k", ci, g)))
                for tt in range(4):
                    jobs.append((0, lambda ci=ci, tt=tt: vproj(ci, tt)))

            # ---- phase 2: attention; deferred jobs drain inside key loops
            for lc in range(NCH):
                ls = slice(lc * 512, (lc + 1) * 512)
                for g in range(2):
                    ctx = [
                        ppc.tile([128, 512], F32, tag="ctx", name=f"ctx{h}")
                        for h in range(2)
                    ]
                    cidx = lc * 2 + g
                    for pt in range(NT):
                        # drain deferred jobs BEFORE this pt's consumers are
                        # traced: a vma/km/qm write traced after its reader
                        # would not be seen as a RAW dependency by Tile
                        tick = cidx * NT + pt
                        budget = 2 if cidx == 0 else 1
                        while budget and jobs and jobs[0][0] <= tick:
                            jobs.popleft()[1]()
                            budget -= 1
                        ks = slice(pt * 128, (pt + 1) * 128)
                        squad = ppq.tile([128, 2, 512], F32, tag="sq")
                        for h in range(2):
                            hs = slice(h * 64, (h + 1) * 64)
                            nc.tensor.matmul(
                                squad[:, h, :],
                                lhsT=km[hs, g, ks],
                                rhs=qm[hs, g, ls],
                                tile_position=(h * 64, 0),
                            )
                        es = esp.tile([128, 2, 512], BF16, tag="es")
                        nc.scalar.activation(
                            es.rearrange("p a b -> p (a b)"),
                            squad.rearrange("p a b -> p (a b)"),
                            mybir.ActivationFunctionType.Exp,
                            scale=SCALE,
                        )
                        for h in range(2):
                            nc.tensor.matmul(
                                ctx[h][0:65, :],
                                lhsT=vma[
                                    :, pt, (g * 2 + h) * 66 : (g * 2 + h) * 66 + 65
                                ],
                                rhs=es[:, h, :],
                                start=(pt == 0),
                                stop=(pt == NT - 1),
                            )
                    last = lc == NCH - 1 and g == 1
                    # evacuate ctx to SBUF (frees the PSUM banks) and start
                    # the reciprocal; the normalize runs as a later job
                    cu = normp.tile([65, 2, 512], F32, tag="cu")
                    if last:
                        # quartered so the tail chain starts sooner
                        for qtr in range(4):
                            for h in range(2):
                                nc.vector.tensor_copy(
                                    cu[:, h, qtr * 128 : (qtr + 1) * 128],
                                    ctx[h][0:65, qtr * 128 : (qtr + 1) * 128],
                                )
                    else:
                        for h in range(2):
                            nc.vector.tensor_copy(cu[:, h, :], ctx[h][0:65, :])
                    # 1/d as exp(-ln(d)) on ACT: Ln and Exp share one table
                    # set, this runs in ACT's chunk-boundary idle window
                    # instead of 3.9us on the DVE, and both heads' rows are
                    # adjacent so one instruction pair covers them
                    lnd = normp.tile([1, 2, 512], F32, tag="lnd")
                    rc = normp.tile([1, 2, 512], BF16, tag="rc")
                    if not last:
                        nc.scalar.activation(
                            lnd, cu[64:65, :, :], mybir.ActivationFunctionType.Ln
                        )
                        nc.scalar.activation(
                            rc, lnd, mybir.ActivationFunctionType.Exp, scale=-1.0
                        )
                    nxt = (cidx + 1) * NT
                    if g == 0 and lc < NCH - 1:
                        for fn in qproj_jobs(lc + 1):
                            jobs.append((0, fn))
                    if not last:
                        for h in range(2):
                            jobs.append(
                                (
                                    nxt + 4 + h,
                                    lambda g=g, lc=lc, h=h, cu=cu, rc=rc: (
                                        emit_normalize(g, lc, h, cu, rc)
                                    ),
                                )
                            )
                    if g == 1 and lc < NCH - 1:
                        for tt in range(4):
                            jobs.append(
                                (
                                    nxt + 7 + tt,
                                    lambda lc=lc, tt=tt: emit_outproj(lc, tt),
                                )
                            )
            while jobs:
                jobs.popleft()[1]()
            # tail: quarter-pipelined normalize + out-projection of the
            # last chunk so the reciprocal latency overlaps the out DMAs
            for qtr in range(4):
                q0, q1 = qtr * 128, (qtr + 1) * 128
                nc.scalar.activation(
                    lnd[:, :, q0:q1],
                    cu[64:65, :, q0:q1],
                    mybir.ActivationFunctionType.Ln,
                )
                nc.scalar.activation(
                    rc[:, :, q0:q1],
                    lnd[:, :, q0:q1],
                    mybir.ActivationFunctionType.Exp,
                    scale=-1.0,
                )
                for h in range(2):
                    emit_normalize(1, NCH - 1, h, cu, rc, q0, q1)
                emit_outproj(NCH - 1, qtr)
    return _split_matmul_waits(nc)


_NC_CACHE = None


def kernel(q, k, v, attention_mask, Wq, bq, Wk, bk, Wv, bv, Wo, bo):
    global _NC_CACHE
    q, k, v = (np.asarray(x, np.float32) for x in (q, k, v))
    assert np.asarray(attention_mask).all(), "kernel assumes all-ones mask"
    if _NC_CACHE is None:
        _NC_CACHE = build_nc()
    nc = _NC_CACHE

    bfc = lambda x: np.ascontiguousarray(np.asarray(x, ml_dtypes.bfloat16))
    c = np.ascontiguousarray
    Wq, Wk, Wv, Wo = (np.asarray(x, np.float32) for x in (Wq, Wk, Wv, Wo))
    bq, bk, bv, bo = (np.asarray(x, np.float32) for x in (bq, bk, bv, bo))

    qT = [bfc(q[b].T) for b in range(B)]
    kT = [bfc(k[b].T) for b in range(B)]
    vT = [bfc(v[b].T) for b in range(B)]

    in_maps = []
    for ci in range(N_CORES):
        b = ci // (N_CORES // B)
        j = ci % (N_CORES // B)
        hs = slice(j * CW, (j + 1) * CW)
        in_maps.append(
            {
                "qT": qT[b],
                "kT": kT[b],
                "vT": vT[b],
                "wqt": bfc(Wq.T[:, hs]),
                "wkt": bfc(Wk.T[:, hs]),
                "wvt": bfc(Wv.T[:, hs]),
                "wot": bfc(Wo.T[hs, :]),
                "bq": c(bq[hs, None]),
                "bk": c(bk[hs, None]),
                "bvb": c(np.broadcast_to(bv[hs][None, :], (128, CW))),
            }
        )

    res = bass_utils.run_bass_kernel_spmd(
        nc,
        in_maps,
        core_ids=list(range(N_CORES)),
        tmpdir=os.environ.get("KERNEL_TMPDIR"),
    )
    globals()["LAST_RES"] = res
    out = np.zeros((B, L, D), np.float32)
    for ci, r in enumerate(res.results):
        out[ci // (N_CORES // B)] += r["out_p"]
    out += bo[None, None, :]
    return out
